# revision 36
# baseline (speedup 1.0000x reference)
"""Trainium2 Bass kernel for nn_GatedMultiHeadGATLayer (gnn_message_passing).

V2 design (8 NeuronCores, SPMD single NEFF):
- Nodes remapped (degree-stratified round-robin) into 320 blocks of 128;
  core c owns blocks [40c, 40c+40) (contiguous 5120-row shards).
- Edges sharded by dst block; per block a K=ceil(maxE/128)-chunk slot grid
  of 128-edge chunks, split lo/hi by src row with an overlap window
  ([0,32768) / [8192,40960)) so both halves balance to K/2 chunks and
  gather indices fit int16.
- Node tables are fp8: layer1 row = [asrc4 f32 16B | 4x(z_h fp8 128 + one
  + pad3) | pad] stride 768; layer2 row = [a2src f32 4B | z2 fp8 128 | one
  | pad] stride 256. AllGathered between phases.
- Aggregation: per (chunk, head) matmul psum += selx^T @ row-slice where
  selx = sel (host-precomputed 0/1, streamed) * ex (edge softmax numerator)
  built on DVE in bf16; the row's ones-column accumulates the softmax
  denominator in the same matmul (n=129).
- Per-edge a_dst terms are precomputed for all blocks (selT const stream x
  adb matmuls) overlapping the AllGather; m_w/edge_w are folded into a
  host-precomputed pde table (m_w>0 fast path).
- GRU uses tanh-only gates (sigmoid via tanh identity) to avoid act-table
  reloads; gh = h@Whh^T precomputed per block into SBUF.
"""
import sys, os

sys.path.insert(0, "/opt/trn_rl_repo")
DEBUG_DUMPS = os.environ.get("GAT_DEBUG", "0") == "1"

import numpy as np

import concourse.bass as bass
import concourse.bacc as bacc
import concourse.tile as tile
import concourse.mybir as mybir
from concourse import bass_utils

N = 40000
E = 640000
DIM = 128
HEADS = 4
NCORES = 8
TOTB = 320
B = TOTB // NCORES        # blocks per core (40)
PN = B * 128              # nodes per core (5120)
NP = TOTB * 128           # padded node count (40960)
CHUNKS = [10, 10, 10, 10]  # AllGather chunk sizes in local blocks
CB0 = [0, 10, 20, 30]     # local block start of each chunk
GB0 = [0, 10240, 20480, 30720]  # global table row base of each chunk
LO_END = 30720            # lo window = chunks 0-2
HB = 10240                # hi window base (chunks 1-3)
LA = 8                    # lo-gather lookahead (covers last-AG latency)
ROW1 = 768                # layer1 table row stride bytes
ROW2 = 256                # layer2 table row stride bytes
Z1OFF = 16                # asrc4 f32 in [0,16); head h z at 16+132h
HSTRIDE = 132             # z_h(128) + one(1) + pad(3)
Z2OFF = 4                 # a2src f32 in [0,4); z2 at [4,132); one at 132

f32 = mybir.dt.float32
bf16 = mybir.dt.bfloat16
fp8 = mybir.dt.float8e4
i16 = mybir.dt.int16
AF = mybir.ActivationFunctionType
ALU = mybir.AluOpType


def _pack_edges(src, dst, pd, o2n, trow):
    """Slot grid + host-side selection constants. trow = chunk-major table
    row per node (gather index space); o2n = block/slot id (dst grouping)."""
    nsrc = trow[src]
    ndst = o2n[dst]
    eblk = ndst >> 7
    dloc = ndst & 127
    cat = np.where(nsrc < HB, 0, np.where(nsrc < LO_END, 1, 2))
    order = np.argsort(eblk * 4 + cat, kind="stable")
    eb_s = eblk[order]
    cnt = np.bincount(eblk, minlength=TOTB)
    c0 = np.bincount(eblk[cat == 0], minlength=TOTB)
    c2 = np.bincount(eblk[cat == 2], minlength=TOTB)
    K = int(-(-cnt.max() // 128))
    K_lo = (K + 1) // 2
    K_hi = K - K_lo
    cap_lo, cap_hi = K_lo * 128, K_hi * 128
    lo_cnt = np.clip((cnt + 1) // 2, np.maximum(c0, cnt - cap_hi),
                     np.minimum(cap_lo, cnt - c2))
    assert (lo_cnt >= c0).all() and (cnt - lo_cnt >= c2).all()
    assert (lo_cnt <= cap_lo).all() and (cnt - lo_cnt <= cap_hi).all()

    bstart = np.zeros(TOTB, np.int64)
    bstart[1:] = np.cumsum(cnt)[:-1]
    prel = np.arange(E) - bstart[eb_s]
    half = (prel >= lo_cnt[eb_s]).astype(np.int64)
    rank = np.where(half == 0, prel, prel - lo_cnt[eb_s])
    p = rank % 128
    j = rank // 128 + half * K_lo
    e = order
    idxval = np.where(half == 0, nsrc[e], nsrc[e] - HB)

    idxg = np.zeros((TOTB, 128, K), np.int32)
    pdeg = np.zeros((TOTB, 128, K), np.float32)
    selC = np.zeros((128, TOTB, 128, K), np.uint8)
    idxg[eb_s, p, j] = idxval
    pdeg[eb_s, p, j] = pd[e, 0]
    selC[p, eb_s, dloc[e], j] = 1

    def pack16(mat):  # [TOTB, S] int -> [128, TOTB, S//16] int16
        S = mat.shape[1]
        b_ = mat.astype(np.int16).reshape(TOTB, S // 16, 16).transpose(2, 0, 1)
        return np.ascontiguousarray(np.tile(b_, (8, 1, 1)))

    ilo = pack16(idxg[:, :, :K_lo].transpose(0, 2, 1).reshape(TOTB, cap_lo))
    ihi = pack16(idxg[:, :, K_lo:].transpose(0, 2, 1).reshape(TOTB, cap_hi))
    return ilo, ihi, pdeg, selC, K, K_lo, K_hi


def _build_nc(K, K_lo, K_hi, mwpos, mw_l, mw2pos):
    nc = bacc.Bacc("TRN2", target_bir_lowering=False, debug=False,
                   num_devices=NCORES)
    # ---- I/O ----
    h_sl = nc.dram_tensor("h_sl", [PN, DIM], f32, kind="ExternalInput")
    idxlo = nc.dram_tensor("idxlo", [128, B, 8 * K_lo], i16,
                           kind="ExternalInput")
    idxhi = nc.dram_tensor("idxhi", [128, B, 8 * K_hi], i16,
                           kind="ExternalInput")
    pde4_i = nc.dram_tensor("pde4", [128, B, HEADS, K], bf16,
                            kind="ExternalInput")
    pde2_i = nc.dram_tensor("pde2", [128, B, K], bf16, kind="ExternalInput")
    selC_i = nc.dram_tensor("selC", [128, B, 128, K], bf16,
                            kind="ExternalInput")
    selTC_i = nc.dram_tensor("selTC", [128, B, K, 128], fp8,
                             kind="ExternalInput")
    ident_i = nc.dram_tensor("ident", [128, 128], bf16, kind="ExternalInput")
    fcWT_i = nc.dram_tensor("fcWT", [128, 4 * DIM], bf16,
                            kind="ExternalInput")
    attnp_i = nc.dram_tensor("attnp", [128, 8], bf16, kind="ExternalInput")
    WT2_i = nc.dram_tensor("WT2", [128, HEADS, DIM], bf16,
                           kind="ExternalInput")
    attn2_i = nc.dram_tensor("attn2", [128, 2], bf16, kind="ExternalInput")
    WihT_i = nc.dram_tensor("WihT", [128, 3 * DIM], bf16,
                            kind="ExternalInput")
    WhhT_i = nc.dram_tensor("WhhT", [128, 3 * DIM], bf16,
                            kind="ExternalInput")
    brz_i = nc.dram_tensor("brz", [128, 2 * DIM], bf16, kind="ExternalInput")
    bihn_i = nc.dram_tensor("bihn", [128, DIM], bf16, kind="ExternalInput")
    bhhn_i = nc.dram_tensor("bhhn", [128, DIM], bf16, kind="ExternalInput")
    out_sl = nc.dram_tensor("out_sl", [PN, DIM], f32, kind="ExternalOutput")
    # ---- internal DRAM ----
    Tz1_sl = nc.dram_tensor("Tz1_sl", [PN, ROW1], fp8, kind="Internal")
    Tz1 = nc.dram_tensor("Tz1", [NP, ROW1], fp8, kind="Internal",
                         addr_space="Shared")
    T2_sl = nc.dram_tensor("T2_sl", [PN, ROW2], fp8, kind="Internal")
    xdbg = nc.dram_tensor("xdbg", [PN, 512], f32, kind="Internal")
    asrdbg = nc.dram_tensor("asrdbg", [128, B, HEADS, K], f32,
                            kind="Internal")
    exdbg = nc.dram_tensor("exdbg", [128, B, HEADS, K], f32, kind="Internal")
    addbg = nc.dram_tensor("addbg", [128, B, HEADS, K], f32, kind="Internal")
    T2 = nc.dram_tensor("T2", [NP, ROW2], fp8, kind="Internal",
                        addr_space="Shared")

    rg = [list(range(NCORES))]
    with tile.TileContext(nc) as tc:
        with (
            tc.tile_pool(name="const", bufs=1) as cp,
            tc.tile_pool(name="res", bufs=1) as rp,
            tc.tile_pool(name="zlo", bufs=LA + 2) as zlop,
            tc.tile_pool(name="zhi", bufs=3) as zhip,
            tc.tile_pool(name="selp", bufs=2) as selp,
            tc.tile_pool(name="sxp", bufs=2) as sxp,
            tc.tile_pool(name="work", bufs=2) as wp,
            tc.tile_pool(name="psbig", bufs=2, space="PSUM") as psbig,
            tc.tile_pool(name="psacc", bufs=1, space="PSUM") as psacc,
            tc.tile_pool(name="pstp", bufs=2, space="PSUM") as pstp,
            tc.tile_pool(name="psz2", bufs=1, space="PSUM") as psz2,
            tc.tile_pool(name="pssm", bufs=1, space="PSUM") as pssm,
        ):
            def cload(t_in, shape, dtype):
                t = cp.tile(shape, dtype, tag=t_in.name)
                nc.sync.dma_start(out=t[(slice(None),) * len(shape)],
                                  in_=t_in[(slice(None),) * len(shape)])
                return t

            ident = cload(ident_i, [128, 128], bf16)
            fcWT = cload(fcWT_i, [128, 4 * DIM], bf16)
            attnp = cload(attnp_i, [128, 8], bf16)
            WT2 = cload(WT2_i, [128, HEADS, DIM], bf16)
            attn2 = cload(attn2_i, [128, 2], bf16)
            WihT = cload(WihT_i, [128, 3 * DIM], bf16)
            WhhT = cload(WhhT_i, [128, 3 * DIM], bf16)
            brz = cload(brz_i, [128, 2 * DIM], bf16)
            bihn = cload(bihn_i, [128, DIM], bf16)
            bhhn = cload(bhhn_i, [128, DIM], bf16)
            idxlo_t = cload(idxlo, [128, B, 8 * K_lo], i16)
            idxhi_t = cload(idxhi, [128, B, 8 * K_hi], i16)
            pde4_t = cload(pde4_i, [128, B, HEADS, K], bf16)
            pde2_t = cload(pde2_i, [128, B, K], bf16)
            # residents written on-device
            hTs = rp.tile([128, B, 128], bf16, tag="hTs")
            adbs = rp.tile([128, B, HEADS], bf16, tag="adbs")
            a2bs = rp.tile([128, B, 1], bf16, tag="a2bs")
            adps1 = rp.tile([128, B, HEADS, K], bf16, tag="adps1")
            adps2 = rp.tile([128, B, K], bf16, tag="adps2")

            def ag_chunk(g, src_sl, dst_full):
                r0 = CB0[g] * 128
                r1 = r0 + CHUNKS[g] * 128
                g0 = GB0[g]
                g1 = g0 + CHUNKS[g] * 128 * NCORES
                nc.gpsimd.collective_compute(
                    "AllGather", ALU.bypass, replica_groups=rg,
                    ins=[src_sl[r0:r1, :]], outs=[dst_full[g0:g1, :]])
            # last chunk's trigger is deferred into the NEXT phase's loop
            # (after its first LA lo-gathers) so they can't wait on it
            AG_AT = {CB0[g] + CHUNKS[g]: g for g in range(len(CHUNKS) - 1)}
            LAST_G = len(CHUNKS) - 1

            # ===== phase 0: per-node z / asrc / adst (sw-pipelined) ========
            def p0_prep(b):
                rows = slice(b * 128, (b + 1) * 128)
                hb = wp.tile([128, 128], f32, tag="hblk")
                nc.sync.dma_start(out=hb[:], in_=h_sl[rows, :])
                hbb = wp.tile([128, 128], bf16, tag="hbb")
                nc.vector.tensor_copy(out=hbb[:], in_=hb[:])
                tp = pstp.tile([128, 128], bf16, space="PSUM", tag="tp")
                nc.tensor.transpose(out=tp[:], in_=hbb[:], identity=ident[:])
                nc.scalar.copy(out=hTs[:, b, :], in_=tp[:])
                pz = psbig.tile([128, 512], f32, space="PSUM", tag="big")
                nc.tensor.matmul(pz[:], lhsT=hTs[:, b, :], rhs=fcWT[:],
                                 start=True, stop=True)
                return pz

            def p0_body(b, pz):
                rows = slice(b * 128, (b + 1) * 128)
                t1row = wp.tile([128, ROW1], fp8, tag="t1row")
                t1v = t1row[:, Z1OFF:Z1OFF + HEADS * HSTRIDE].rearrange(
                    "p (h q) -> p h q", q=HSTRIDE)
                nc.vector.memset(t1v[:, :, 128:129], 1.0)
                nc.scalar.activation(t1v[:, :, 0:128],
                                     pz[:].rearrange("p (h d) -> p h d",
                                                     d=128),
                                     AF.Prelu, alpha=0.01)
                z4f = wp.tile([128, 512], bf16, tag="z4f")
                nc.scalar.activation(z4f[:], pz[:], AF.Prelu, alpha=0.01)
                a8ps = pssm.tile([128, 8], f32, space="PSUM", tag="sm")
                for h4 in range(HEADS):
                    tph = pstp.tile([128, 128], bf16, space="PSUM", tag="tp")
                    nc.tensor.transpose(out=tph[:],
                                        in_=z4f[:, h4 * 128:(h4 + 1) * 128],
                                        identity=ident[:])
                    z4T = wp.tile([128, 128], bf16, tag="z4T")
                    if h4 % 2 == 0:
                        nc.scalar.copy(out=z4T[:], in_=tph[:])
                    else:
                        nc.vector.tensor_copy(out=z4T[:], in_=tph[:])
                    nc.tensor.matmul(a8ps[:, 2 * h4:2 * h4 + 2], lhsT=z4T[:],
                                     rhs=attnp[:, 2 * h4:2 * h4 + 2],
                                     start=True, stop=True)
                a8v = a8ps[:].rearrange("p (four two) -> p four two", two=2)
                nc.vector.tensor_copy(
                    out=t1row[:, 0:Z1OFF].bitcast(f32), in_=a8v[:, :, 0])
                nc.scalar.copy(out=adbs[:, b, :], in_=a8v[:, :, 1])
                nc.sync.dma_start(out=Tz1_sl[rows, :], in_=t1row[:])

            p0_st = None
            for b in range(B + 1):
                if b < B:
                    nxt0 = p0_prep(b)
                if p0_st is not None:
                    p0_body(b - 1, p0_st)
                    if b in AG_AT:
                        ag_chunk(AG_AT[b], Tz1_sl, Tz1)
                p0_st = nxt0 if b < B else None

            # ---- precompute1: per-edge adst for all blocks (overlaps AG1)
            for b in range(B):
                selT_t = selp.tile([128, K, 128], fp8, tag="selT")
                nc.sync.dma_start(out=selT_t[:, :, :], in_=selTC_i[:, b, :, :])
                aps = pssm.tile([128, K, HEADS], f32, space="PSUM", tag="sm")
                for j in range(K):
                    nc.tensor.matmul(aps[:, j, :], lhsT=selT_t[:, j, :],
                                     rhs=adbs[:, b, :], start=True, stop=True)
                nc.vector.tensor_copy(
                    out=adps1[:, b, :, :],
                    in_=aps[:].rearrange("p k h -> p h k"))

            # ================= layer 1 edge phase (sw-pipelined) ===========
            def l1_lo(b):
                zlo = zlop.tile([128, K_lo, ROW1], fp8, tag="zlo")
                nc.gpsimd.dma_gather(
                    out_ap=zlo[:, :, :], in_ap=Tz1[0:LO_END, :],
                    idxs_ap=idxlo_t[:, b, :], num_idxs=K_lo * 128,
                    num_idxs_reg=K_lo * 128, elem_size=ROW1,
                    single_packet=False)
                return zlo

            def l1_hi(b, zlo):
                zhi = zhip.tile([128, K_hi, ROW1], fp8, tag="zhi")
                nc.gpsimd.dma_gather(
                    out_ap=zhi[:, :, :], in_ap=Tz1[HB:, :],
                    idxs_ap=idxhi_t[:, b, :], num_idxs=K_hi * 128,
                    num_idxs_reg=K_hi * 128, elem_size=ROW1,
                    single_packet=False)
                selb = selp.tile([128, 128, K], bf16, tag="selb")
                nc.sync.dma_start(out=selb[:, :, :], in_=selC_i[:, b, :, :])
                asr = wp.tile([128, HEADS, K], f32, tag="asr")
                nc.vector.tensor_copy(
                    out=asr[:, :, 0:K_lo],
                    in_=zlo[:, :, 0:Z1OFF].bitcast(f32).rearrange(
                        "p k h -> p h k"))
                nc.vector.tensor_copy(
                    out=asr[:, :, K_lo:],
                    in_=zhi[:, :, 0:Z1OFF].bitcast(f32).rearrange(
                        "p k h -> p h k"))
                nc.vector.tensor_tensor(out=asr[:], in0=asr[:],
                                        in1=adps1[:, b, :, :], op=ALU.add)
                nc.vector.tensor_tensor(out=asr[:], in0=asr[:],
                                        in1=pde4_t[:, b, :, :], op=ALU.mult)
                ex4 = wp.tile([128, HEADS, K], bf16, tag="ex4")
                if mwpos:
                    nc.scalar.activation(ex4[:], asr[:], AF.Prelu, alpha=0.01)
                    nc.scalar.activation(ex4[:], ex4[:], AF.Exp)
                else:
                    for h4 in range(HEADS):
                        sc, al = ((1.0, 0.01) if mw_l[h4] > 0
                                  else (0.01, 100.0))
                        nc.scalar.activation(ex4[:, h4, :], asr[:, h4, :],
                                             AF.Prelu, scale=sc, alpha=al)
                    nc.scalar.activation(ex4[:], ex4[:], AF.Exp)
                selx = sxp.tile([128, 128, HEADS, K], bf16, tag="selx")
                for pas in range(2):
                    hs = slice(2 * pas, 2 * pas + 2)
                    nc.vector.tensor_tensor(
                        out=selx[:, :, hs, :],
                        in0=selb[:, :, None, :].to_broadcast([128, 128, 2, K]),
                        in1=ex4[:, None, hs, :].to_broadcast([128, 128, 2, K]),
                        op=ALU.mult)
                return zlo, zhi, selx

            def l1_body(b, zlo, zhi, selx):
                rows = slice(b * 128, (b + 1) * 128)
                xb = wp.tile([128, 512], bf16, tag="xb")
                for pas in range(2):
                    psA = psacc.tile([128, 129], f32, space="PSUM", tag="accA")
                    psB = psacc.tile([128, 129], f32, space="PSUM", tag="accB")
                    for j in range(K):
                        zt, jj = (zlo, j) if j < K_lo else (zhi, j - K_lo)
                        zv = zt[:, jj,
                                Z1OFF:Z1OFF + HEADS * HSTRIDE].rearrange(
                            "p (h q) -> p h q", q=HSTRIDE)
                        nc.tensor.matmul(
                            psA[:], lhsT=selx[:, :, 2 * pas, j],
                            rhs=zv[:, 2 * pas, 0:129],
                            start=(j == 0), stop=(j == K - 1))
                        nc.tensor.matmul(
                            psB[:], lhsT=selx[:, :, 2 * pas + 1, j],
                            rhs=zv[:, 2 * pas + 1, 0:129],
                            start=(j == 0), stop=(j == K - 1))
                    den = wp.tile([128, 2], f32, tag=f"den_{pas}")
                    nc.vector.tensor_scalar(out=den[:, 0:1],
                                            in0=psA[:, 128:129],
                                            scalar1=1e-30, scalar2=None,
                                            op0=ALU.max)
                    nc.vector.tensor_scalar(out=den[:, 1:2],
                                            in0=psB[:, 128:129],
                                            scalar1=1e-30, scalar2=None,
                                            op0=ALU.max)
                    r2 = wp.tile([128, 2], f32, tag=f"r2_{pas}")
                    nc.vector.reciprocal(out=r2[:], in_=den[:])
                    nc.scalar.activation(
                        xb[:, (2 * pas) * 128:(2 * pas + 1) * 128],
                        psA[:, 0:128], AF.Prelu,
                        scale=r2[:, 0:1], alpha=0.01)
                    nc.scalar.activation(
                        xb[:, (2 * pas + 1) * 128:(2 * pas + 2) * 128],
                        psB[:, 0:128], AF.Prelu,
                        scale=r2[:, 1:2], alpha=0.01)
                xT = wp.tile([128, 512], bf16, tag="xT")
                for h4 in range(HEADS):
                    tpq = pstp.tile([128, 128], bf16, space="PSUM", tag="tp")
                    nc.tensor.transpose(out=tpq[:],
                                        in_=xb[:, h4 * 128:(h4 + 1) * 128],
                                        identity=ident[:])
                    if h4 % 2 == 0:
                        nc.scalar.copy(out=xT[:, h4 * 128:(h4 + 1) * 128],
                                       in_=tpq[:])
                    else:
                        nc.vector.tensor_copy(
                            out=xT[:, h4 * 128:(h4 + 1) * 128], in_=tpq[:])
                z2d = psz2.tile([128, 128], f32, space="PSUM", tag="z2d")
                for h4 in range(HEADS):
                    nc.tensor.matmul(z2d[:], lhsT=xT[:, h4 * 128:(h4 + 1) * 128],
                                     rhs=WT2[:, h4, :], start=(h4 == 0),
                                     stop=(h4 == 3))
                t2row = wp.tile([128, ROW2], fp8, tag="t2row")
                nc.vector.memset(t2row[:, Z2OFF + 128:Z2OFF + 129], 1.0)
                nc.scalar.activation(t2row[:, Z2OFF:Z2OFF + 128], z2d[:],
                                     AF.Prelu, alpha=0.01)
                for h4 in range(HEADS):
                    nc.tensor.matmul(z2d[:], lhsT=WT2[:, h4, :],
                                     rhs=xT[:, h4 * 128:(h4 + 1) * 128],
                                     start=(h4 == 0), stop=(h4 == 3))
                z2Tl = wp.tile([128, 128], bf16, tag="z2Tl")
                nc.scalar.activation(z2Tl[:], z2d[:], AF.Prelu, alpha=0.01)
                a2ps = pssm.tile([128, 2], f32, space="PSUM", tag="sm")
                nc.tensor.matmul(a2ps[:], lhsT=z2Tl[:], rhs=attn2[:],
                                 start=True, stop=True)
                nc.vector.tensor_copy(out=t2row[:, 0:Z2OFF].bitcast(f32),
                                      in_=a2ps[:, 0:1])
                nc.vector.tensor_copy(out=a2bs[:, b, :], in_=a2ps[:, 1:2])
                nc.sync.dma_start(out=T2_sl[rows, :], in_=t2row[:])

            lo_q = {}
            l1_st = None
            for t in range(B + LA + 1):
                if t < B:
                    lo_q[t] = l1_lo(t)
                if t == LA - 1:
                    ag_chunk(LAST_G, Tz1_sl, Tz1)  # deferred AG1 last chunk
                b2 = t - LA
                nxt = l1_hi(b2, lo_q.pop(b2)) if 0 <= b2 < B else None
                if l1_st is not None:
                    l1_body(b2 - 1, *l1_st)
                    if b2 in AG_AT:
                        ag_chunk(AG_AT[b2], T2_sl, T2)
                l1_st = nxt

            # ---- precompute2: per-edge a2dst (overlaps AG2)
            for b in range(B):
                selT_t = selp.tile([128, K, 128], fp8, tag="selT")
                nc.sync.dma_start(out=selT_t[:, :, :], in_=selTC_i[:, b, :, :])
                aps = pssm.tile([128, K, 1], f32, space="PSUM", tag="sm")
                for j in range(K):
                    nc.tensor.matmul(aps[:, j, :], lhsT=selT_t[:, j, :],
                                     rhs=a2bs[:, b, :], start=True, stop=True)
                nc.vector.tensor_copy(out=adps2[:, b, :], in_=aps[:, :, 0])

            # ========= layer 2 edge phase + GRU (sw-pipelined) =============
            # L2 gathers land in the L1 zlo/zhi tile rings (ROW2-sized view
            # of the front of each ROW1-sized tile) to avoid extra SBUF.
            def rw2(t, kk):
                flat = t[:].rearrange("p k q -> p (k q)")
                return flat[:, 0:kk * ROW2].rearrange("p (k q) -> p k q",
                                                      q=ROW2)

            def l2_lo(b):
                zlot = zlop.tile([128, K_lo, ROW1], fp8, tag="zlo")
                zlo = rw2(zlot, K_lo)
                nc.gpsimd.dma_gather(
                    out_ap=zlo[:, :, :], in_ap=T2[0:LO_END, :],
                    idxs_ap=idxlo_t[:, b, :], num_idxs=K_lo * 128,
                    num_idxs_reg=K_lo * 128, elem_size=ROW2,
                    single_packet=False)
                return zlo

            def l2_hi(b, zlo):
                zhit = zhip.tile([128, K_hi, ROW1], fp8, tag="zhi")
                zhi = rw2(zhit, K_hi)
                nc.gpsimd.dma_gather(
                    out_ap=zhi[:, :, :], in_ap=T2[HB:, :],
                    idxs_ap=idxhi_t[:, b, :], num_idxs=K_hi * 128,
                    num_idxs_reg=K_hi * 128, elem_size=ROW2,
                    single_packet=False)
                selb = selp.tile([128, 128, K], bf16, tag="selb")
                nc.sync.dma_start(out=selb[:, :, :], in_=selC_i[:, b, :, :])
                asr = wp.tile([128, K], f32, tag="asr2")
                nc.vector.tensor_copy(
                    out=asr[:, 0:K_lo],
                    in_=zlo[:, :, 0:Z2OFF].bitcast(f32)[:, :, 0])
                nc.vector.tensor_copy(
                    out=asr[:, K_lo:],
                    in_=zhi[:, :, 0:Z2OFF].bitcast(f32)[:, :, 0])
                nc.vector.tensor_tensor(out=asr[:], in0=asr[:],
                                        in1=adps2[:, b, :], op=ALU.add)
                nc.vector.tensor_tensor(out=asr[:], in0=asr[:],
                                        in1=pde2_t[:, b, :], op=ALU.mult)
                ex2 = wp.tile([128, K], bf16, tag="ex2")
                sc, al = (1.0, 0.01) if mw2pos else (0.01, 100.0)
                nc.scalar.activation(ex2[:], asr[:], AF.Prelu, scale=sc,
                                     alpha=al)
                nc.scalar.activation(ex2[:], ex2[:], AF.Exp)
                selx = sxp.tile([128, 128, K], bf16, tag="selx2")
                nc.vector.tensor_tensor(
                    out=selx[:, :, :],
                    in0=selb[:, :, :],
                    in1=ex2[:, None, :].to_broadcast([128, 128, K]),
                    op=ALU.mult)
                return zlo, zhi, selx

            def l2_body(b, zlo, zhi, selx):
                rows = slice(b * 128, (b + 1) * 128)
                ps2 = psacc.tile([128, 129], f32, space="PSUM", tag="accA")
                for j in range(K):
                    zt, jj = (zlo, j) if j < K_lo else (zhi, j - K_lo)
                    nc.tensor.matmul(ps2[:], lhsT=selx[:, :, j],
                                     rhs=zt[:, jj, Z2OFF:Z2OFF + 129],
                                     start=(j == 0), stop=(j == K - 1))
                den = wp.tile([128, 1], f32, tag="den2")
                nc.vector.tensor_scalar(out=den[:], in0=ps2[:, 128:129],
                                        scalar1=1e-30, scalar2=None,
                                        op0=ALU.max)
                r1 = wp.tile([128, 1], f32, tag="r1")
                nc.vector.reciprocal(out=r1[:], in_=den[:])
                x2 = wp.tile([128, 128], bf16, tag="x2")
                nc.scalar.activation(x2[:], ps2[:, 0:128], AF.Prelu,
                                     scale=r1[:, 0:1], alpha=0.01)
                tpx = pstp.tile([128, 128], bf16, space="PSUM", tag="tp")
                nc.tensor.transpose(out=tpx[:], in_=x2[:], identity=ident[:])
                x2T = wp.tile([128, 128], bf16, tag="x2T")
                nc.scalar.copy(out=x2T[:], in_=tpx[:])
                hb = wp.tile([128, 128], f32, tag="hblk2")
                nc.sync.dma_start(out=hb[:], in_=h_sl[rows, :])
                # giPs[:, 0:256] accumulates BOTH x2@Wih^T and h@Whh^T for
                # the r/z gates; [256:384] holds gi_n alone. gh_n separate.
                giPs = psbig.tile([128, 384], f32, space="PSUM", tag="big")
                nc.tensor.matmul(giPs[:], lhsT=x2T[:], rhs=WihT[:],
                                 start=True, stop=False, skip_group_check=True)
                nc.tensor.matmul(giPs[:, 0:256], lhsT=hTs[:, b, :],
                                 rhs=WhhT[:, 0:256], start=False, stop=True,
                                 skip_group_check=True)
                ghnp = psz2.tile([128, 128], f32, space="PSUM", tag="z2d")
                nc.tensor.matmul(ghnp[:], lhsT=hTs[:, b, :],
                                 rhs=WhhT[:, 256:384], start=True, stop=True)
                # GRU with tanh-only gates: sigm(u) = 0.5*tanh(u/2)+0.5
                Trz = wp.tile([128, 256], bf16, tag="Trz")
                nc.vector.tensor_tensor(out=Trz[:], in0=giPs[:, 0:256],
                                        in1=brz[:], op=ALU.add)
                rg_ = wp.tile([128, 128], bf16, tag="rg")
                nc.scalar.activation(rg_[:], Trz[:, 0:128], AF.Tanh,
                                     scale=0.5)
                zg = wp.tile([128, 128], bf16, tag="zgate")
                nc.scalar.activation(zg[:], Trz[:, 128:256], AF.Tanh,
                                     scale=0.5)
                gin = wp.tile([128, 128], bf16, tag="gin")
                nc.vector.tensor_tensor(out=gin[:], in0=giPs[:, 256:384],
                                        in1=bihn[:], op=ALU.add)
                ghn = wp.tile([128, 128], bf16, tag="ghn")
                nc.vector.tensor_tensor(out=ghn[:], in0=ghnp[:],
                                        in1=bhhn[:], op=ALU.add)
                t_ = wp.tile([128, 128], bf16, tag="tmul")
                nc.vector.tensor_tensor(out=t_[:], in0=rg_[:], in1=ghn[:],
                                        op=ALU.mult)
                nc.vector.tensor_tensor(out=t_[:], in0=t_[:], in1=ghn[:],
                                        op=ALU.add)
                nc.vector.tensor_scalar(out=t_[:], in0=t_[:], scalar1=0.5,
                                        scalar2=None, op0=ALU.mult)
                nc.vector.tensor_tensor(out=t_[:], in0=t_[:], in1=gin[:],
                                        op=ALU.add)
                ng = wp.tile([128, 128], bf16, tag="ng")
                nc.scalar.activation(ng[:], t_[:], AF.Tanh)
                c_ = wp.tile([128, 128], bf16, tag="cdiff")
                nc.vector.tensor_tensor(out=c_[:], in0=hb[:], in1=ng[:],
                                        op=ALU.subtract)
                d_ = wp.tile([128, 128], bf16, tag="dmul")
                nc.vector.tensor_tensor(out=d_[:], in0=zg[:], in1=c_[:],
                                        op=ALU.mult)
                nc.vector.tensor_tensor(out=d_[:], in0=d_[:], in1=c_[:],
                                        op=ALU.add)
                f_ = wp.tile([128, 128], f32, tag="fout")
                nc.vector.tensor_scalar(out=f_[:], in0=d_[:], scalar1=0.5,
                                        scalar2=None, op0=ALU.mult)
                nc.vector.tensor_tensor(out=f_[:], in0=f_[:], in1=ng[:],
                                        op=ALU.add)
                nt = wp.tile([128, 128], f32, tag="nt")
                nc.scalar.activation(nt[:], f_[:], AF.Prelu, alpha=0.01)
                nc.sync.dma_start(out=out_sl[rows, :], in_=nt[:])

            lo2_q = {}
            l2_st = None
            for t in range(B + LA + 1):
                if t < B:
                    lo2_q[t] = l2_lo(t)
                if t == LA - 1:
                    ag_chunk(LAST_G, T2_sl, T2)  # deferred AG2 last chunk
                b2 = t - LA
                nxt2 = l2_hi(b2, lo2_q.pop(b2)) if 0 <= b2 < B else None
                if l2_st is not None:
                    l2_body(b2 - 1, *l2_st)
                l2_st = nxt2
    nc.finalize()
    return nc


def kernel(h, pd, fc_W, attn_W, edge_w, m_w, out_fc_W, out_attn_W, out_edge_w,
           out_m_w, gru_Wih, gru_Whh, gru_bih, gru_bhh, src, dst):
    import ml_dtypes
    npbf16 = np.dtype(ml_dtypes.bfloat16)
    npfp8 = np.dtype(ml_dtypes.float8_e4m3fn)
    h = np.asarray(h, np.float32)
    pd = np.asarray(pd, np.float32)
    src = np.asarray(src, np.int64)
    dst = np.asarray(dst, np.int64)
    deg = np.bincount(dst, minlength=N)
    order = np.argsort(-deg, kind="stable")
    o2n = np.empty(N, np.int64)
    o2n[order] = (np.arange(N) % TOTB) * 128 + np.arange(N) // TOTB
    # chunk-major table row (AG chunk g outer, then core, then local block)
    blk = o2n >> 7
    loc = o2n & 127
    cr, bl = blk // B, blk % B
    gidx = np.searchsorted(np.array(CB0), bl, side="right") - 1
    nbg = np.array(CHUNKS)[gidx]
    trow = (np.array(GB0)[gidx] + cr * (nbg * 128)
            + (bl - np.array(CB0)[gidx]) * 128 + loc)
    ilo, ihi, pdeg, selC, K, K_lo, K_hi = _pack_edges(src, dst, pd, o2n, trow)

    ew = np.array([float(edge_w[i, 0, 0]) for i in range(HEADS)])
    mw = np.array([float(m_w[i, 0, 0]) for i in range(HEADS)])
    mwpos = bool((mw > 0).all())
    ew2 = float(out_edge_w[0, 0])
    mw2 = float(out_m_w[0, 0])
    mw2pos = mw2 > 0
    emw = ew * mw
    em2 = ew2 * mw2
    # pde4[p, b, h, j], pde2[p, b, j]
    pde4 = (pdeg.transpose(1, 0, 2)[:, :, None, :]
            * emw[None, None, :, None]).astype(npbf16)
    pde2 = (pdeg.transpose(1, 0, 2) * em2).astype(npbf16)

    h_new = np.zeros((NP, DIM), np.float32)
    h_new[o2n] = h
    fcWT = np.ascontiguousarray(
        np.concatenate([fc_W[i].T for i in range(HEADS)], 1)).astype(npbf16)
    attnp = np.zeros((128, 8), np.float32)  # cast to bf16 below
    for i in range(HEADS):
        attnp[:, 2 * i] = attn_W[i, 0, :DIM]
        attnp[:, 2 * i + 1] = attn_W[i, 0, DIM:]
    WT2 = np.ascontiguousarray(
        np.asarray(out_fc_W, np.float32).reshape(DIM, HEADS, DIM)
        .transpose(2, 1, 0)).astype(npbf16)
    attn2 = np.ascontiguousarray(
        np.stack([out_attn_W[0, :DIM], out_attn_W[0, DIM:]], 1),
        dtype=np.float32).astype(npbf16)
    bsum = (np.asarray(gru_bih, np.float32)
            + np.asarray(gru_bhh, np.float32))
    consts = {
        "ident": np.eye(128, dtype=np.float32).astype(npbf16),
        "fcWT": fcWT, "attnp": attnp.astype(npbf16), "WT2": WT2, "attn2": attn2,
        "WihT": np.ascontiguousarray(np.asarray(gru_Wih, np.float32).T)
        .astype(npbf16),
        "WhhT": np.ascontiguousarray(np.asarray(gru_Whh, np.float32).T)
        .astype(npbf16),
        "brz": np.tile(bsum[None, 0:256], (128, 1)).astype(npbf16),
        "bihn": np.tile(np.asarray(gru_bih, np.float32)[None, 256:384],
                        (128, 1)).astype(npbf16),
        "bhhn": np.tile(np.asarray(gru_bhh, np.float32)[None, 256:384],
                        (128, 1)).astype(npbf16),
    }
    selC_bf = selC.astype(npbf16)
    # selC is [q, TOTB, d, j]; selTC should be [d, TOTB, j, q]
    selT_f8 = np.ascontiguousarray(
        selC.transpose(2, 1, 3, 0)).astype(npfp8)

    nc = _build_nc(K, K_lo, K_hi, mwpos, list(mw), mw2pos)
    in_maps = []
    for c in range(NCORES):
        bs = slice(B * c, B * (c + 1))
        in_maps.append({
            "h_sl": np.ascontiguousarray(h_new[PN * c: PN * (c + 1)]),
            "idxlo": np.ascontiguousarray(ilo[:, bs, :]),
            "idxhi": np.ascontiguousarray(ihi[:, bs, :]),
            "pde4": np.ascontiguousarray(pde4[:, bs, :, :]),
            "pde2": np.ascontiguousarray(pde2[:, bs, :]),
            "selC": np.ascontiguousarray(selC_bf[:, bs, :, :]),
            "selTC": np.ascontiguousarray(selT_f8[:, bs, :, :]),
            **consts,
        })
    res = bass_utils.run_bass_kernel_spmd(nc, in_maps,
                                          core_ids=list(range(NCORES)))
    global _last_results
    _last_results = res
    out_new = np.concatenate([res.results[c]["out_sl"] for c in range(NCORES)])
    return np.ascontiguousarray(out_new[o2n])


_last_results = None



# revision 39
# speedup vs baseline: 1.1759x; 1.1759x over previous
"""Trainium2 Bass kernel for nn_GatedMultiHeadGATLayer (gnn_message_passing).

V2 design (8 NeuronCores, SPMD single NEFF):
- Nodes remapped (degree-stratified round-robin) into 320 blocks of 128;
  core c owns blocks [40c, 40c+40) (contiguous 5120-row shards).
- Edges sharded by dst block; per block a K=ceil(maxE/128)-chunk slot grid
  of 128-edge chunks, split lo/hi by src row with an overlap window
  ([0,32768) / [8192,40960)) so both halves balance to K/2 chunks and
  gather indices fit int16.
- Node tables are fp8: layer1 row = [asrc4 f32 16B | 4x(z_h fp8 128 + one
  + pad3) | pad] stride 768; layer2 row = [a2src f32 4B | z2 fp8 128 | one
  | pad] stride 256. AllGathered between phases.
- Aggregation: per (chunk, head) matmul psum += selx^T @ row-slice where
  selx = sel (host-precomputed 0/1, streamed) * ex (edge softmax numerator)
  built on DVE in bf16; the row's ones-column accumulates the softmax
  denominator in the same matmul (n=129).
- Per-edge a_dst terms are precomputed for all blocks (selT const stream x
  adb matmuls) overlapping the AllGather; m_w/edge_w are folded into a
  host-precomputed pde table (m_w>0 fast path).
- GRU uses tanh-only gates (sigmoid via tanh identity) to avoid act-table
  reloads; gh = h@Whh^T precomputed per block into SBUF.
"""
import sys, os

sys.path.insert(0, "/opt/trn_rl_repo")
DEBUG_DUMPS = os.environ.get("GAT_DEBUG", "0") == "1"

import numpy as np

import concourse.bass as bass
import concourse.bacc as bacc
import concourse.tile as tile
import concourse.mybir as mybir
from concourse import bass_utils

N = 40000
E = 640000
DIM = 128
HEADS = 4
NCORES = 8
TOTB = 320
B = TOTB // NCORES        # blocks per core (40)
PN = B * 128              # nodes per core (5120)
NP = TOTB * 128           # padded node count (40960)
CHUNKS = [10, 10, 10, 10]  # AllGather chunk sizes in local blocks
CB0 = [0, 10, 20, 30]     # local block start of each chunk
GB0 = [0, 10240, 20480, 30720]  # global table row base of each chunk
LO_END = 30720            # lo window = chunks 0-2
HB = 10240                # hi window base (chunks 1-3)
LA = 8                    # lo-gather lookahead (covers last-AG latency)
ROW1 = 768                # layer1 table row stride bytes
ROW2 = 256                # layer2 table row stride bytes
Z1OFF = 16                # asrc4 f32 in [0,16); head h z at 16+132h
HSTRIDE = 132             # z_h(128) + one(1) + pad(3)
Z2OFF = 4                 # a2src f32 in [0,4); z2 at [4,132); one at 132

f32 = mybir.dt.float32
bf16 = mybir.dt.bfloat16
fp8 = mybir.dt.float8e4
i16 = mybir.dt.int16
AF = mybir.ActivationFunctionType
ALU = mybir.AluOpType


def _pack_edges(src, dst, pd, o2n, trow):
    """Slot grid + host-side selection constants. trow = chunk-major table
    row per node (gather index space); o2n = block/slot id (dst grouping)."""
    nsrc = trow[src]
    ndst = o2n[dst]
    eblk = ndst >> 7
    dloc = ndst & 127
    cat = np.where(nsrc < HB, 0, np.where(nsrc < LO_END, 1, 2))
    order = np.argsort(eblk * 4 + cat, kind="stable")
    eb_s = eblk[order]
    cnt = np.bincount(eblk, minlength=TOTB)
    c0 = np.bincount(eblk[cat == 0], minlength=TOTB)
    c2 = np.bincount(eblk[cat == 2], minlength=TOTB)
    K = int(-(-cnt.max() // 128))
    K_lo = (K + 1) // 2
    K_hi = K - K_lo
    cap_lo, cap_hi = K_lo * 128, K_hi * 128
    lo_cnt = np.clip((cnt + 1) // 2, np.maximum(c0, cnt - cap_hi),
                     np.minimum(cap_lo, cnt - c2))
    assert (lo_cnt >= c0).all() and (cnt - lo_cnt >= c2).all()
    assert (lo_cnt <= cap_lo).all() and (cnt - lo_cnt <= cap_hi).all()

    bstart = np.zeros(TOTB, np.int64)
    bstart[1:] = np.cumsum(cnt)[:-1]
    prel = np.arange(E) - bstart[eb_s]
    half = (prel >= lo_cnt[eb_s]).astype(np.int64)
    rank = np.where(half == 0, prel, prel - lo_cnt[eb_s])
    p = rank % 128
    j = rank // 128 + half * K_lo
    e = order
    idxval = np.where(half == 0, nsrc[e], nsrc[e] - HB)

    idxg = np.zeros((TOTB, 128, K), np.int32)
    pdeg = np.zeros((TOTB, 128, K), np.float32)
    selC = np.zeros((128, TOTB, 128, K), np.uint8)
    idxg[eb_s, p, j] = idxval
    pdeg[eb_s, p, j] = pd[e, 0]
    selC[p, eb_s, dloc[e], j] = 1

    def pack16(mat):  # [TOTB, S] int -> [128, TOTB, S//16] int16
        S = mat.shape[1]
        b_ = mat.astype(np.int16).reshape(TOTB, S // 16, 16).transpose(2, 0, 1)
        return np.ascontiguousarray(np.tile(b_, (8, 1, 1)))

    ilo = pack16(idxg[:, :, :K_lo].transpose(0, 2, 1).reshape(TOTB, cap_lo))
    ihi = pack16(idxg[:, :, K_lo:].transpose(0, 2, 1).reshape(TOTB, cap_hi))
    return ilo, ihi, pdeg, selC, K, K_lo, K_hi


def _build_nc(K, K_lo, K_hi, mwpos, mw_l, mw2pos):
    nc = bacc.Bacc("TRN2", target_bir_lowering=False, debug=False,
                   num_devices=NCORES)
    # ---- I/O ----
    h_sl = nc.dram_tensor("h_sl", [PN, DIM], f32, kind="ExternalInput")
    idxlo = nc.dram_tensor("idxlo", [128, B, 8 * K_lo], i16,
                           kind="ExternalInput")
    idxhi = nc.dram_tensor("idxhi", [128, B, 8 * K_hi], i16,
                           kind="ExternalInput")
    pde4_i = nc.dram_tensor("pde4", [128, B, HEADS, K], bf16,
                            kind="ExternalInput")
    pde2_i = nc.dram_tensor("pde2", [128, B, K], bf16, kind="ExternalInput")
    selC_i = nc.dram_tensor("selC", [128, B, 128, K], bf16,
                            kind="ExternalInput")
    selTC_i = nc.dram_tensor("selTC", [128, B, K, 128], fp8,
                             kind="ExternalInput")
    ident_i = nc.dram_tensor("ident", [128, 128], bf16, kind="ExternalInput")
    fcWT_i = nc.dram_tensor("fcWT", [128, 4 * DIM], bf16,
                            kind="ExternalInput")
    attnp_i = nc.dram_tensor("attnp", [128, 8], bf16, kind="ExternalInput")
    WT2_i = nc.dram_tensor("WT2", [128, HEADS, DIM], bf16,
                           kind="ExternalInput")
    attn2_i = nc.dram_tensor("attn2", [128, 2], bf16, kind="ExternalInput")
    WihT_i = nc.dram_tensor("WihT", [128, 3 * DIM], bf16,
                            kind="ExternalInput")
    WhhT_i = nc.dram_tensor("WhhT", [128, 3 * DIM], bf16,
                            kind="ExternalInput")
    brz_i = nc.dram_tensor("brz", [128, 2 * DIM], bf16, kind="ExternalInput")
    bihn_i = nc.dram_tensor("bihn", [128, DIM], bf16, kind="ExternalInput")
    bhhn_i = nc.dram_tensor("bhhn", [128, DIM], bf16, kind="ExternalInput")
    out_sl = nc.dram_tensor("out_sl", [PN, DIM], f32, kind="ExternalOutput")
    # ---- internal DRAM ----
    Tz1_sl = nc.dram_tensor("Tz1_sl", [PN, ROW1], fp8, kind="Internal")
    Tz1 = nc.dram_tensor("Tz1", [NP, ROW1], fp8, kind="Internal",
                         addr_space="Shared")
    T2_sl = nc.dram_tensor("T2_sl", [PN, ROW2], fp8, kind="Internal")
    xdbg = nc.dram_tensor("xdbg", [PN, 512], f32, kind="Internal")
    asrdbg = nc.dram_tensor("asrdbg", [128, B, HEADS, K], f32,
                            kind="Internal")
    exdbg = nc.dram_tensor("exdbg", [128, B, HEADS, K], f32, kind="Internal")
    addbg = nc.dram_tensor("addbg", [128, B, HEADS, K], f32, kind="Internal")
    T2 = nc.dram_tensor("T2", [NP, ROW2], fp8, kind="Internal",
                        addr_space="Shared")

    rg = [list(range(NCORES))]
    with tile.TileContext(nc) as tc:
        with (
            tc.tile_pool(name="const", bufs=1) as cp,
            tc.tile_pool(name="res", bufs=1) as rp,
            tc.tile_pool(name="zlo", bufs=LA + 2) as zlop,
            tc.tile_pool(name="zhi", bufs=3) as zhip,
            tc.tile_pool(name="selp", bufs=2) as selp,
            tc.tile_pool(name="sxp", bufs=2) as sxp,
            tc.tile_pool(name="work", bufs=2) as wp,
            tc.tile_pool(name="psbig", bufs=2, space="PSUM") as psbig,
            tc.tile_pool(name="psacc", bufs=1, space="PSUM") as psacc,
            tc.tile_pool(name="pstp", bufs=2, space="PSUM") as pstp,
            tc.tile_pool(name="psz2", bufs=1, space="PSUM") as psz2,
            tc.tile_pool(name="pssm", bufs=1, space="PSUM") as pssm,
        ):
            def cload(t_in, shape, dtype):
                t = cp.tile(shape, dtype, tag=t_in.name)
                nc.sync.dma_start(out=t[(slice(None),) * len(shape)],
                                  in_=t_in[(slice(None),) * len(shape)])
                return t

            ident = cload(ident_i, [128, 128], bf16)
            fcWT = cload(fcWT_i, [128, 4 * DIM], bf16)
            attnp = cload(attnp_i, [128, 8], bf16)
            WT2 = cload(WT2_i, [128, HEADS, DIM], bf16)
            attn2 = cload(attn2_i, [128, 2], bf16)
            WihT = cload(WihT_i, [128, 3 * DIM], bf16)
            WhhT = cload(WhhT_i, [128, 3 * DIM], bf16)
            brz = cload(brz_i, [128, 2 * DIM], bf16)
            bihn = cload(bihn_i, [128, DIM], bf16)
            bhhn = cload(bhhn_i, [128, DIM], bf16)
            idxlo_t = cload(idxlo, [128, B, 8 * K_lo], i16)
            idxhi_t = cload(idxhi, [128, B, 8 * K_hi], i16)
            pde4_t = cload(pde4_i, [128, B, HEADS, K], bf16)
            pde2_t = cload(pde2_i, [128, B, K], bf16)
            # residents written on-device
            hTs = rp.tile([128, B, 128], bf16, tag="hTs")
            adbs = rp.tile([128, B, HEADS], bf16, tag="adbs")
            a2bs = rp.tile([128, B, 1], bf16, tag="a2bs")
            adps1 = rp.tile([128, B, HEADS, K], bf16, tag="adps1")
            adps2 = rp.tile([128, B, K], bf16, tag="adps2")

            def ag_chunk(g, src_sl, dst_full):
                r0 = CB0[g] * 128
                r1 = r0 + CHUNKS[g] * 128
                g0 = GB0[g]
                g1 = g0 + CHUNKS[g] * 128 * NCORES
                nc.gpsimd.collective_compute(
                    "AllGather", ALU.bypass, replica_groups=rg,
                    ins=[src_sl[r0:r1, :]], outs=[dst_full[g0:g1, :]])
            AG_AT = {CB0[g] + CHUNKS[g]: g for g in range(len(CHUNKS))}

            # ===== phase 0: per-node z / asrc / adst (sw-pipelined) ========
            def p0_prep(b):
                rows = slice(b * 128, (b + 1) * 128)
                hb = wp.tile([128, 128], f32, tag="hblk")
                nc.sync.dma_start(out=hb[:], in_=h_sl[rows, :])
                hbb = wp.tile([128, 128], bf16, tag="hbb")
                nc.vector.tensor_copy(out=hbb[:], in_=hb[:])
                tp = pstp.tile([128, 128], bf16, space="PSUM", tag="tp")
                nc.tensor.transpose(out=tp[:], in_=hbb[:], identity=ident[:])
                nc.scalar.copy(out=hTs[:, b, :], in_=tp[:])
                pz = psbig.tile([128, 512], f32, space="PSUM", tag="big")
                nc.tensor.matmul(pz[:], lhsT=hTs[:, b, :], rhs=fcWT[:],
                                 start=True, stop=True)
                return pz

            def p0_body(b, pz):
                rows = slice(b * 128, (b + 1) * 128)
                t1row = wp.tile([128, ROW1], fp8, tag="t1row")
                t1v = t1row[:, Z1OFF:Z1OFF + HEADS * HSTRIDE].rearrange(
                    "p (h q) -> p h q", q=HSTRIDE)
                nc.vector.memset(t1v[:, :, 128:129], 1.0)
                nc.scalar.activation(t1v[:, :, 0:128],
                                     pz[:].rearrange("p (h d) -> p h d",
                                                     d=128),
                                     AF.Prelu, alpha=0.01)
                z4f = wp.tile([128, 512], bf16, tag="z4f")
                nc.scalar.activation(z4f[:], pz[:], AF.Prelu, alpha=0.01)
                a8ps = pssm.tile([128, 8], f32, space="PSUM", tag="sm")
                for h4 in range(HEADS):
                    tph = pstp.tile([128, 128], bf16, space="PSUM", tag="tp")
                    nc.tensor.transpose(out=tph[:],
                                        in_=z4f[:, h4 * 128:(h4 + 1) * 128],
                                        identity=ident[:])
                    z4T = wp.tile([128, 128], bf16, tag="z4T")
                    if h4 % 2 == 0:
                        nc.scalar.copy(out=z4T[:], in_=tph[:])
                    else:
                        nc.vector.tensor_copy(out=z4T[:], in_=tph[:])
                    nc.tensor.matmul(a8ps[:, 2 * h4:2 * h4 + 2], lhsT=z4T[:],
                                     rhs=attnp[:, 2 * h4:2 * h4 + 2],
                                     start=True, stop=True)
                a8v = a8ps[:].rearrange("p (four two) -> p four two", two=2)
                nc.vector.tensor_copy(
                    out=t1row[:, 0:Z1OFF].bitcast(f32), in_=a8v[:, :, 0])
                nc.scalar.copy(out=adbs[:, b, :], in_=a8v[:, :, 1])
                nc.sync.dma_start(out=Tz1_sl[rows, :], in_=t1row[:])

            p0_st = None
            for b in range(B + 1):
                if b < B:
                    nxt0 = p0_prep(b)
                if p0_st is not None:
                    p0_body(b - 1, p0_st)
                    if b in AG_AT:
                        ag_chunk(AG_AT[b], Tz1_sl, Tz1)
                p0_st = nxt0 if b < B else None

            # ---- precompute1: per-edge adst for all blocks (overlaps AG1)
            for b in range(B):
                selT_t = selp.tile([128, K, 128], fp8, tag="selT")
                nc.sync.dma_start(out=selT_t[:, :, :], in_=selTC_i[:, b, :, :])
                aps = pssm.tile([128, K, HEADS], f32, space="PSUM", tag="sm")
                for j in range(K):
                    nc.tensor.matmul(aps[:, j, :], lhsT=selT_t[:, j, :],
                                     rhs=adbs[:, b, :], start=True, stop=True)
                nc.vector.tensor_copy(
                    out=adps1[:, b, :, :],
                    in_=aps[:].rearrange("p k h -> p h k"))

            # ================= layer 1 edge phase (sw-pipelined) ===========
            def l1_lo(b):
                zlo = zlop.tile([128, K_lo, ROW1], fp8, tag="zlo")
                nc.gpsimd.dma_gather(
                    out_ap=zlo[:, :, :], in_ap=Tz1[0:LO_END, :],
                    idxs_ap=idxlo_t[:, b, :], num_idxs=K_lo * 128,
                    num_idxs_reg=K_lo * 128, elem_size=ROW1,
                    single_packet=False)
                return zlo

            def l1_hi(b, zlo):
                zhi = zhip.tile([128, K_hi, ROW1], fp8, tag="zhi")
                nc.gpsimd.dma_gather(
                    out_ap=zhi[:, :, :], in_ap=Tz1[HB:, :],
                    idxs_ap=idxhi_t[:, b, :], num_idxs=K_hi * 128,
                    num_idxs_reg=K_hi * 128, elem_size=ROW1,
                    single_packet=False)
                selb = selp.tile([128, 128, K], bf16, tag="selb")
                nc.sync.dma_start(out=selb[:, :, :], in_=selC_i[:, b, :, :])
                asr = wp.tile([128, HEADS, K], f32, tag="asr")
                nc.vector.tensor_copy(
                    out=asr[:, :, 0:K_lo],
                    in_=zlo[:, :, 0:Z1OFF].bitcast(f32).rearrange(
                        "p k h -> p h k"))
                nc.vector.tensor_copy(
                    out=asr[:, :, K_lo:],
                    in_=zhi[:, :, 0:Z1OFF].bitcast(f32).rearrange(
                        "p k h -> p h k"))
                nc.vector.tensor_tensor(out=asr[:], in0=asr[:],
                                        in1=adps1[:, b, :, :], op=ALU.add)
                nc.vector.tensor_tensor(out=asr[:], in0=asr[:],
                                        in1=pde4_t[:, b, :, :], op=ALU.mult)
                ex4 = wp.tile([128, HEADS, K], bf16, tag="ex4")
                if mwpos:
                    nc.scalar.activation(ex4[:], asr[:], AF.Prelu, alpha=0.01)
                    nc.scalar.activation(ex4[:], ex4[:], AF.Exp)
                else:
                    for h4 in range(HEADS):
                        sc, al = ((1.0, 0.01) if mw_l[h4] > 0
                                  else (0.01, 100.0))
                        nc.scalar.activation(ex4[:, h4, :], asr[:, h4, :],
                                             AF.Prelu, scale=sc, alpha=al)
                    nc.scalar.activation(ex4[:], ex4[:], AF.Exp)
                selx = sxp.tile([128, 128, HEADS, K], bf16, tag="selx")
                for pas in range(2):
                    hs = slice(2 * pas, 2 * pas + 2)
                    nc.vector.tensor_tensor(
                        out=selx[:, :, hs, :],
                        in0=selb[:, :, None, :].to_broadcast([128, 128, 2, K]),
                        in1=ex4[:, None, hs, :].to_broadcast([128, 128, 2, K]),
                        op=ALU.mult)
                return zlo, zhi, selx

            def l1_body(b, zlo, zhi, selx):
                rows = slice(b * 128, (b + 1) * 128)
                xb = wp.tile([128, 512], bf16, tag="xb")
                for pas in range(2):
                    psA = psacc.tile([128, 129], f32, space="PSUM", tag="accA")
                    psB = psacc.tile([128, 129], f32, space="PSUM", tag="accB")
                    for j in range(K):
                        zt, jj = (zlo, j) if j < K_lo else (zhi, j - K_lo)
                        zv = zt[:, jj,
                                Z1OFF:Z1OFF + HEADS * HSTRIDE].rearrange(
                            "p (h q) -> p h q", q=HSTRIDE)
                        nc.tensor.matmul(
                            psA[:], lhsT=selx[:, :, 2 * pas, j],
                            rhs=zv[:, 2 * pas, 0:129],
                            start=(j == 0), stop=(j == K - 1))
                        nc.tensor.matmul(
                            psB[:], lhsT=selx[:, :, 2 * pas + 1, j],
                            rhs=zv[:, 2 * pas + 1, 0:129],
                            start=(j == 0), stop=(j == K - 1))
                    den = wp.tile([128, 2], f32, tag=f"den_{pas}")
                    nc.vector.tensor_scalar(out=den[:, 0:1],
                                            in0=psA[:, 128:129],
                                            scalar1=1e-30, scalar2=None,
                                            op0=ALU.max)
                    nc.vector.tensor_scalar(out=den[:, 1:2],
                                            in0=psB[:, 128:129],
                                            scalar1=1e-30, scalar2=None,
                                            op0=ALU.max)
                    r2 = wp.tile([128, 2], f32, tag=f"r2_{pas}")
                    nc.vector.reciprocal(out=r2[:], in_=den[:])
                    nc.scalar.activation(
                        xb[:, (2 * pas) * 128:(2 * pas + 1) * 128],
                        psA[:, 0:128], AF.Prelu,
                        scale=r2[:, 0:1], alpha=0.01)
                    nc.scalar.activation(
                        xb[:, (2 * pas + 1) * 128:(2 * pas + 2) * 128],
                        psB[:, 0:128], AF.Prelu,
                        scale=r2[:, 1:2], alpha=0.01)
                xT = wp.tile([128, 512], bf16, tag="xT")
                for h4 in range(HEADS):
                    tpq = pstp.tile([128, 128], bf16, space="PSUM", tag="tp")
                    nc.tensor.transpose(out=tpq[:],
                                        in_=xb[:, h4 * 128:(h4 + 1) * 128],
                                        identity=ident[:])
                    if h4 % 2 == 0:
                        nc.scalar.copy(out=xT[:, h4 * 128:(h4 + 1) * 128],
                                       in_=tpq[:])
                    else:
                        nc.vector.tensor_copy(
                            out=xT[:, h4 * 128:(h4 + 1) * 128], in_=tpq[:])
                z2d = psz2.tile([128, 128], f32, space="PSUM", tag="z2d")
                for h4 in range(HEADS):
                    nc.tensor.matmul(z2d[:], lhsT=xT[:, h4 * 128:(h4 + 1) * 128],
                                     rhs=WT2[:, h4, :], start=(h4 == 0),
                                     stop=(h4 == 3))
                t2row = wp.tile([128, ROW2], fp8, tag="t2row")
                nc.vector.memset(t2row[:, Z2OFF + 128:Z2OFF + 129], 1.0)
                nc.scalar.activation(t2row[:, Z2OFF:Z2OFF + 128], z2d[:],
                                     AF.Prelu, alpha=0.01)
                for h4 in range(HEADS):
                    nc.tensor.matmul(z2d[:], lhsT=WT2[:, h4, :],
                                     rhs=xT[:, h4 * 128:(h4 + 1) * 128],
                                     start=(h4 == 0), stop=(h4 == 3))
                z2Tl = wp.tile([128, 128], bf16, tag="z2Tl")
                nc.scalar.activation(z2Tl[:], z2d[:], AF.Prelu, alpha=0.01)
                a2ps = pssm.tile([128, 2], f32, space="PSUM", tag="sm")
                nc.tensor.matmul(a2ps[:], lhsT=z2Tl[:], rhs=attn2[:],
                                 start=True, stop=True)
                nc.vector.tensor_copy(out=t2row[:, 0:Z2OFF].bitcast(f32),
                                      in_=a2ps[:, 0:1])
                nc.vector.tensor_copy(out=a2bs[:, b, :], in_=a2ps[:, 1:2])
                nc.sync.dma_start(out=T2_sl[rows, :], in_=t2row[:])

            lo_q = {}
            l1_st = None
            for t in range(B + LA + 1):
                if t < B:
                    lo_q[t] = l1_lo(t)
                b2 = t - LA
                nxt = l1_hi(b2, lo_q.pop(b2)) if 0 <= b2 < B else None
                if l1_st is not None:
                    l1_body(b2 - 1, *l1_st)
                    if b2 in AG_AT:
                        ag_chunk(AG_AT[b2], T2_sl, T2)
                l1_st = nxt

            # ---- precompute2: per-edge a2dst (overlaps AG2)
            for b in range(B):
                selT_t = selp.tile([128, K, 128], fp8, tag="selT")
                nc.sync.dma_start(out=selT_t[:, :, :], in_=selTC_i[:, b, :, :])
                aps = pssm.tile([128, K, 1], f32, space="PSUM", tag="sm")
                for j in range(K):
                    nc.tensor.matmul(aps[:, j, :], lhsT=selT_t[:, j, :],
                                     rhs=a2bs[:, b, :], start=True, stop=True)
                nc.vector.tensor_copy(out=adps2[:, b, :], in_=aps[:, :, 0])

            # ========= layer 2 edge phase + GRU (sw-pipelined) =============
            # L2 gathers land in the L1 zlo/zhi tile rings (ROW2-sized view
            # of the front of each ROW1-sized tile) to avoid extra SBUF.
            def rw2(t, kk):
                flat = t[:].rearrange("p k q -> p (k q)")
                return flat[:, 0:kk * ROW2].rearrange("p (k q) -> p k q",
                                                      q=ROW2)

            def l2_lo(b):
                zlot = zlop.tile([128, K_lo, ROW1], fp8, tag="zlo")
                zlo = rw2(zlot, K_lo)
                nc.gpsimd.dma_gather(
                    out_ap=zlo[:, :, :], in_ap=T2[0:LO_END, :],
                    idxs_ap=idxlo_t[:, b, :], num_idxs=K_lo * 128,
                    num_idxs_reg=K_lo * 128, elem_size=ROW2,
                    single_packet=False)
                return zlo

            def l2_hi(b, zlo):
                zhit = zhip.tile([128, K_hi, ROW1], fp8, tag="zhi")
                zhi = rw2(zhit, K_hi)
                nc.gpsimd.dma_gather(
                    out_ap=zhi[:, :, :], in_ap=T2[HB:, :],
                    idxs_ap=idxhi_t[:, b, :], num_idxs=K_hi * 128,
                    num_idxs_reg=K_hi * 128, elem_size=ROW2,
                    single_packet=False)
                selb = selp.tile([128, 128, K], bf16, tag="selb")
                nc.sync.dma_start(out=selb[:, :, :], in_=selC_i[:, b, :, :])
                asr = wp.tile([128, K], f32, tag="asr2")
                nc.vector.tensor_copy(
                    out=asr[:, 0:K_lo],
                    in_=zlo[:, :, 0:Z2OFF].bitcast(f32)[:, :, 0])
                nc.vector.tensor_copy(
                    out=asr[:, K_lo:],
                    in_=zhi[:, :, 0:Z2OFF].bitcast(f32)[:, :, 0])
                nc.vector.tensor_tensor(out=asr[:], in0=asr[:],
                                        in1=adps2[:, b, :], op=ALU.add)
                nc.vector.tensor_tensor(out=asr[:], in0=asr[:],
                                        in1=pde2_t[:, b, :], op=ALU.mult)
                ex2 = wp.tile([128, K], bf16, tag="ex2")
                sc, al = (1.0, 0.01) if mw2pos else (0.01, 100.0)
                nc.scalar.activation(ex2[:], asr[:], AF.Prelu, scale=sc,
                                     alpha=al)
                nc.scalar.activation(ex2[:], ex2[:], AF.Exp)
                selx = sxp.tile([128, 128, K], bf16, tag="selx2")
                nc.vector.tensor_tensor(
                    out=selx[:, :, :],
                    in0=selb[:, :, :],
                    in1=ex2[:, None, :].to_broadcast([128, 128, K]),
                    op=ALU.mult)
                return zlo, zhi, selx

            def l2_body(b, zlo, zhi, selx):
                rows = slice(b * 128, (b + 1) * 128)
                ps2 = psacc.tile([128, 129], f32, space="PSUM", tag="accA")
                for j in range(K):
                    zt, jj = (zlo, j) if j < K_lo else (zhi, j - K_lo)
                    nc.tensor.matmul(ps2[:], lhsT=selx[:, :, j],
                                     rhs=zt[:, jj, Z2OFF:Z2OFF + 129],
                                     start=(j == 0), stop=(j == K - 1))
                den = wp.tile([128, 1], f32, tag="den2")
                nc.vector.tensor_scalar(out=den[:], in0=ps2[:, 128:129],
                                        scalar1=1e-30, scalar2=None,
                                        op0=ALU.max)
                r1 = wp.tile([128, 1], f32, tag="r1")
                nc.vector.reciprocal(out=r1[:], in_=den[:])
                x2 = wp.tile([128, 128], bf16, tag="x2")
                nc.scalar.activation(x2[:], ps2[:, 0:128], AF.Prelu,
                                     scale=r1[:, 0:1], alpha=0.01)
                tpx = pstp.tile([128, 128], bf16, space="PSUM", tag="tp")
                nc.tensor.transpose(out=tpx[:], in_=x2[:], identity=ident[:])
                x2T = wp.tile([128, 128], bf16, tag="x2T")
                nc.scalar.copy(out=x2T[:], in_=tpx[:])
                hb = wp.tile([128, 128], f32, tag="hblk2")
                nc.sync.dma_start(out=hb[:], in_=h_sl[rows, :])
                # giPs[:, 0:256] accumulates BOTH x2@Wih^T and h@Whh^T for
                # the r/z gates; [256:384] holds gi_n alone. gh_n separate.
                giPs = psbig.tile([128, 384], f32, space="PSUM", tag="big")
                nc.tensor.matmul(giPs[:], lhsT=x2T[:], rhs=WihT[:],
                                 start=True, stop=False, skip_group_check=True)
                nc.tensor.matmul(giPs[:, 0:256], lhsT=hTs[:, b, :],
                                 rhs=WhhT[:, 0:256], start=False, stop=True,
                                 skip_group_check=True)
                ghnp = psz2.tile([128, 128], f32, space="PSUM", tag="z2d")
                nc.tensor.matmul(ghnp[:], lhsT=hTs[:, b, :],
                                 rhs=WhhT[:, 256:384], start=True, stop=True)
                # GRU with tanh-only gates: sigm(u) = 0.5*tanh(u/2)+0.5
                Trz = wp.tile([128, 256], bf16, tag="Trz")
                nc.vector.tensor_tensor(out=Trz[:], in0=giPs[:, 0:256],
                                        in1=brz[:], op=ALU.add)
                rg_ = wp.tile([128, 128], bf16, tag="rg")
                nc.scalar.activation(rg_[:], Trz[:, 0:128], AF.Tanh,
                                     scale=0.5)
                zg = wp.tile([128, 128], bf16, tag="zgate")
                nc.scalar.activation(zg[:], Trz[:, 128:256], AF.Tanh,
                                     scale=0.5)
                gin = wp.tile([128, 128], bf16, tag="gin")
                nc.vector.tensor_tensor(out=gin[:], in0=giPs[:, 256:384],
                                        in1=bihn[:], op=ALU.add)
                ghn = wp.tile([128, 128], bf16, tag="ghn")
                nc.vector.tensor_tensor(out=ghn[:], in0=ghnp[:],
                                        in1=bhhn[:], op=ALU.add)
                t_ = wp.tile([128, 128], bf16, tag="tmul")
                nc.vector.tensor_tensor(out=t_[:], in0=rg_[:], in1=ghn[:],
                                        op=ALU.mult)
                nc.vector.tensor_tensor(out=t_[:], in0=t_[:], in1=ghn[:],
                                        op=ALU.add)
                nc.vector.tensor_scalar(out=t_[:], in0=t_[:], scalar1=0.5,
                                        scalar2=None, op0=ALU.mult)
                nc.vector.tensor_tensor(out=t_[:], in0=t_[:], in1=gin[:],
                                        op=ALU.add)
                ng = wp.tile([128, 128], bf16, tag="ng")
                nc.scalar.activation(ng[:], t_[:], AF.Tanh)
                c_ = wp.tile([128, 128], bf16, tag="cdiff")
                nc.vector.tensor_tensor(out=c_[:], in0=hb[:], in1=ng[:],
                                        op=ALU.subtract)
                d_ = wp.tile([128, 128], bf16, tag="dmul")
                nc.vector.tensor_tensor(out=d_[:], in0=zg[:], in1=c_[:],
                                        op=ALU.mult)
                nc.vector.tensor_tensor(out=d_[:], in0=d_[:], in1=c_[:],
                                        op=ALU.add)
                f_ = wp.tile([128, 128], f32, tag="fout")
                nc.vector.tensor_scalar(out=f_[:], in0=d_[:], scalar1=0.5,
                                        scalar2=None, op0=ALU.mult)
                nc.vector.tensor_tensor(out=f_[:], in0=f_[:], in1=ng[:],
                                        op=ALU.add)
                nt = wp.tile([128, 128], f32, tag="nt")
                nc.scalar.activation(nt[:], f_[:], AF.Prelu, alpha=0.01)
                nc.sync.dma_start(out=out_sl[rows, :], in_=nt[:])

            lo2_q = {}
            l2_st = None
            for t in range(B + LA + 1):
                if t < B:
                    lo2_q[t] = l2_lo(t)
                b2 = t - LA
                nxt2 = l2_hi(b2, lo2_q.pop(b2)) if 0 <= b2 < B else None
                if l2_st is not None:
                    l2_body(b2 - 1, *l2_st)
                l2_st = nxt2
    nc.finalize()
    return nc


def kernel(h, pd, fc_W, attn_W, edge_w, m_w, out_fc_W, out_attn_W, out_edge_w,
           out_m_w, gru_Wih, gru_Whh, gru_bih, gru_bhh, src, dst):
    import ml_dtypes
    npbf16 = np.dtype(ml_dtypes.bfloat16)
    npfp8 = np.dtype(ml_dtypes.float8_e4m3fn)
    h = np.asarray(h, np.float32)
    pd = np.asarray(pd, np.float32)
    src = np.asarray(src, np.int64)
    dst = np.asarray(dst, np.int64)
    deg = np.bincount(dst, minlength=N)
    order = np.argsort(-deg, kind="stable")
    o2n = np.empty(N, np.int64)
    o2n[order] = (np.arange(N) % TOTB) * 128 + np.arange(N) // TOTB
    # chunk-major table row (AG chunk g outer, then core, then local block)
    blk = o2n >> 7
    loc = o2n & 127
    cr, bl = blk // B, blk % B
    gidx = np.searchsorted(np.array(CB0), bl, side="right") - 1
    nbg = np.array(CHUNKS)[gidx]
    trow = (np.array(GB0)[gidx] + cr * (nbg * 128)
            + (bl - np.array(CB0)[gidx]) * 128 + loc)
    ilo, ihi, pdeg, selC, K, K_lo, K_hi = _pack_edges(src, dst, pd, o2n, trow)

    ew = np.array([float(edge_w[i, 0, 0]) for i in range(HEADS)])
    mw = np.array([float(m_w[i, 0, 0]) for i in range(HEADS)])
    mwpos = bool((mw > 0).all())
    ew2 = float(out_edge_w[0, 0])
    mw2 = float(out_m_w[0, 0])
    mw2pos = mw2 > 0
    emw = ew * mw
    em2 = ew2 * mw2
    # pde4[p, b, h, j], pde2[p, b, j]
    pde4 = (pdeg.transpose(1, 0, 2)[:, :, None, :]
            * emw[None, None, :, None]).astype(npbf16)
    pde2 = (pdeg.transpose(1, 0, 2) * em2).astype(npbf16)

    h_new = np.zeros((NP, DIM), np.float32)
    h_new[o2n] = h
    fcWT = np.ascontiguousarray(
        np.concatenate([fc_W[i].T for i in range(HEADS)], 1)).astype(npbf16)
    attnp = np.zeros((128, 8), np.float32)  # cast to bf16 below
    for i in range(HEADS):
        attnp[:, 2 * i] = attn_W[i, 0, :DIM]
        attnp[:, 2 * i + 1] = attn_W[i, 0, DIM:]
    WT2 = np.ascontiguousarray(
        np.asarray(out_fc_W, np.float32).reshape(DIM, HEADS, DIM)
        .transpose(2, 1, 0)).astype(npbf16)
    attn2 = np.ascontiguousarray(
        np.stack([out_attn_W[0, :DIM], out_attn_W[0, DIM:]], 1),
        dtype=np.float32).astype(npbf16)
    bsum = (np.asarray(gru_bih, np.float32)
            + np.asarray(gru_bhh, np.float32))
    consts = {
        "ident": np.eye(128, dtype=np.float32).astype(npbf16),
        "fcWT": fcWT, "attnp": attnp.astype(npbf16), "WT2": WT2, "attn2": attn2,
        "WihT": np.ascontiguousarray(np.asarray(gru_Wih, np.float32).T)
        .astype(npbf16),
        "WhhT": np.ascontiguousarray(np.asarray(gru_Whh, np.float32).T)
        .astype(npbf16),
        "brz": np.tile(bsum[None, 0:256], (128, 1)).astype(npbf16),
        "bihn": np.tile(np.asarray(gru_bih, np.float32)[None, 256:384],
                        (128, 1)).astype(npbf16),
        "bhhn": np.tile(np.asarray(gru_bhh, np.float32)[None, 256:384],
                        (128, 1)).astype(npbf16),
    }
    selC_bf = selC.astype(npbf16)
    # selC is [q, TOTB, d, j]; selTC should be [d, TOTB, j, q]
    selT_f8 = np.ascontiguousarray(
        selC.transpose(2, 1, 3, 0)).astype(npfp8)

    nc = _build_nc(K, K_lo, K_hi, mwpos, list(mw), mw2pos)
    in_maps = []
    for c in range(NCORES):
        bs = slice(B * c, B * (c + 1))
        in_maps.append({
            "h_sl": np.ascontiguousarray(h_new[PN * c: PN * (c + 1)]),
            "idxlo": np.ascontiguousarray(ilo[:, bs, :]),
            "idxhi": np.ascontiguousarray(ihi[:, bs, :]),
            "pde4": np.ascontiguousarray(pde4[:, bs, :, :]),
            "pde2": np.ascontiguousarray(pde2[:, bs, :]),
            "selC": np.ascontiguousarray(selC_bf[:, bs, :, :]),
            "selTC": np.ascontiguousarray(selT_f8[:, bs, :, :]),
            **consts,
        })
    res = bass_utils.run_bass_kernel_spmd(nc, in_maps,
                                          core_ids=list(range(NCORES)))
    global _last_results
    _last_results = res
    out_new = np.concatenate([res.results[c]["out_sl"] for c in range(NCORES)])
    return np.ascontiguousarray(out_new[o2n])


_last_results = None

